# revision 16
# baseline (speedup 1.0000x reference)
"""Multi-Head Latent Attention (naive MLA) on 8 Trainium2 NeuronCores.

Sharding: data-parallel over batch (2) x causal-balanced sequence-parallel
over queries (4-way interleave): core c handles batch b = c//4, query group
g = c%4 (global query rows g, g+4, g+8, ...).  Every core runs the identical
SPMD program; only the data differs.  To keep the program data-independent,
the host rotates each x-column group of 4 so the core's queries sit at
columns 0::4 (keys are therefore mildly permuted within each group of 4;
the wedge-mask matrix, also host data, accounts for that permutation).
No collectives: each core produces the full output rows for its queries.

All matmuls contract over the SBUF partition dim, so everything is kept
"transposed" (feature-major) end to end and no on-device transposes are
needed:
  latentT = Wdkv^T @ x^T                  [128, 2048]  bf16
  qT      = Wq^T @ xT[:, 0::4]            [1024, 512]  bf16
  kT      = Wuk^T @ latentT               [1024, 2048] bf16
  v_aug   = [latent @ Wuv | ones] per key tile          bf16
  scoresT: per head pair, one 2-bank psum [128 keys, 2x512]: parity p's
           scores in columns 512p..; causal wedge added by a rank-32 mask
           matmul; ONE fused exp per bin on ScalarE covers both parities
           (1/sqrt(hd) folded into the activation scale); softmax denom
           comes for free as row 64 of the ctx matmul (ones column of v_aug)
  ctxT_h  = v_aug^T @ expT                [65, 512] psum accum over key tiles
  out     = matmul(lhsT=ctxT tiles, rhs=Wo); bias added by the psum->sbuf
            copy (tensor_tensor with a broadcast bias tile)  -> [512, 1024]

The attention stream is software-pipelined (scores one bin ahead of exp,
exp one ahead of ctx) and the phase-1/2 GEMMs (qT / kT / v_aug chunks) are
emitted BETWEEN attention bins so the PE keeps running while ScalarE does
exp; all psum->sbuf copies run on Pool (gpsimd) / DVE, never ScalarE.
PSUM: psc 2x[128,1024] for scores (4 banks), pctx 2x[65,512] ctx accum
(2 banks), pout 2x[128,512] for background GEMMs / bias / out-proj.
"""

import numpy as np

B, S, D, L, H = 2, 2048, 1024, 128, 16
HD = D // H        # 64
AUG = HD + 1       # 65 (v dims + ones column for softmax denominator)
NCORES = 8
GQ = S // 4        # 512 queries per core
QT = 512           # queries per tile (single tile)
KT = 128           # keys per key tile
NKT = S // KT      # 16
NEG = -640.0       # additive mask pre-exp-scale (x 1/8 -> -80)

_cache = {}


def _worklist(offset):
    """Strip list [(u, cs, wedge)]; identical across cores.

    Query column c = global row 4*c+g, position +offset.  cs (first
    computed column of the strip) uses the worst core (g=3) so strip
    shapes are core-independent; the wedge matrix (data) carries g.
    """
    items = []
    for u in range(NKT):
        lo = KT * u
        min_qpos = 0 + offset
        max_qpos = 4 * (QT - 1) + 3 + offset
        if lo + KT - 1 <= min_qpos:
            items.append((u, 0, False))      # fully allowed
        elif lo > max_qpos:
            continue                         # fully masked: skip
        else:
            cs = max(0, -((-(lo - 3 - offset)) // 4))
            assert 0 <= cs < QT
            items.append((u, cs, True))
    assert items and items[0][1] == 0, "first strip must cover col 0"
    return items


def _wedge_matrix(g, offset, items):
    """[32, 128] f32: T[m, j] = NEG where local key j is masked at strip
    col m.  Host rotates x columns so queries are 0::4; local key index j
    of a strip is global position lo + pi(j), pi(j) = 4*(j//4)+(j%4+g)%4.
    Masked iff lo+pi(j) > q_pos = 4*(cs+m)+g+offset, i.e. pi(j) > 4*m+r0,
    r0 = 4*cs+g+offset-lo (strip-independent; asserted).
    """
    r0s = set()
    for (u, cs, wedge) in items:
        if wedge:
            r0s.add(4 * cs + g + offset - KT * u)
    if not r0s:
        r0s = {g}
    assert len(r0s) == 1, f"non-uniform wedge r0 {r0s} (offset={offset})"
    r0 = r0s.pop()
    assert 0 <= r0 <= 127, r0
    j = np.arange(128)
    pi = 4 * (j // 4) + (j % 4 + g) % 4
    T = np.zeros((32, 128), np.float32)
    for m in range(32):
        T[m, :] = np.where(pi > 4 * m + r0, NEG, 0.0)
    return T


def _blocks_of(items):
    """Pack strips into per-parity psum bins of <=512 cols (one matmul's
    psum output can't cross a bank).  Returns [(list[(item, off)], fill)]."""
    bins = []
    cur, w = [], 0
    for it in items:
        sw = QT - it[1]
        if w + sw > 512:
            bins.append((cur, w))
            cur, w = [], 0
        cur.append((it, w))
        w += sw
    if cur:
        bins.append((cur, w))
    return bins


def _build(offset, reps=1):
    import concourse.bacc as bacc
    import concourse.tile as tile
    import concourse.mybir as mybir
    from contextlib import ExitStack

    f32r = mybir.dt.float32r
    bf16 = mybir.dt.bfloat16
    f32 = mybir.dt.float32

    nc = bacc.Bacc("TRN2", target_bir_lowering=False, debug=False,
                   num_devices=NCORES)
    xT = nc.dram_tensor("xT", [D, S], bf16, kind="ExternalInput").ap()
    Wq = nc.dram_tensor("Wq", [D, D], bf16, kind="ExternalInput").ap()
    Wdkv = nc.dram_tensor("Wdkv", [D, L], bf16, kind="ExternalInput").ap()
    Wukv = nc.dram_tensor("Wukv", [L, 2 * D], bf16, kind="ExternalInput").ap()
    Wo = nc.dram_tensor("Wo", [D, D], bf16, kind="ExternalInput").ap()
    bo = nc.dram_tensor("bo", [1, D], f32r, kind="ExternalInput").ap()
    Twedge = nc.dram_tensor("Twedge", [32, 128], bf16,
                            kind="ExternalInput").ap()
    I32 = nc.dram_tensor("I32", [32, 32], bf16, kind="ExternalInput").ap()
    Ones = nc.dram_tensor("Ones", [1, 130], f32r, kind="ExternalInput").ap()
    out = nc.dram_tensor("out", [GQ, D], f32, kind="ExternalOutput").ap()

    for _rep in range(reps):
        _emit_body(nc, tile, mybir, ExitStack, offset,
                   xT, Wq, Wdkv, Wukv, Wo, bo, Twedge, I32, Ones, out)

    nc.compile()
    return nc


def _emit_body(nc, tile, mybir, ExitStack, offset,
               xT, Wq, Wdkv, Wukv, Wo, bo, Twedge, I32, Ones, out):
    f32r = mybir.dt.float32r
    bf16 = mybir.dt.bfloat16
    f32 = mybir.dt.float32
    AF = mybir.ActivationFunctionType
    ALU = mybir.AluOpType

    items = _worklist(offset)
    bins = _blocks_of(items)
    NB = len(bins)

    with tile.TileContext(nc) as tc, ExitStack() as ctx:
        big = ctx.enter_context(tc.tile_pool(name="big", bufs=1, side="left"))
        sm = ctx.enter_context(tc.tile_pool(name="sm", bufs=1, side="right"))
        sexp = ctx.enter_context(tc.tile_pool(name="sexp", bufs=3,
                                              side="right"))
        sout = ctx.enter_context(tc.tile_pool(name="sout", bufs=2,
                                              side="right"))
        precs = ctx.enter_context(tc.tile_pool(name="precs", bufs=2,
                                               side="right"))
        psc = ctx.enter_context(tc.tile_pool(name="psc", bufs=2,
                                             space="PSUM", side="left"))
        pctx = ctx.enter_context(tc.tile_pool(name="pctx", bufs=2,
                                              space="PSUM", side="right"))
        pout = ctx.enter_context(tc.tile_pool(name="pout", bufs=2,
                                              space="PSUM", side="right"))

        # ------------- input DMAs (dependency-priority order) -------------
        xT_sb = big.tile([128, D // 128, S], bf16, tag="xT")
        Wdkv_sb = big.tile([128, D // 128, L], bf16, tag="Wdkv")
        Wukv_sb = big.tile([128, 2 * D], bf16, tag="Wukv")
        Wq_sb = big.tile([128, D // 128, D], bf16, tag="Wq")
        Wo_sb = big.tile([128, D // 128, D], bf16, tag="Wo")

        def dma_xT(n):
            nc.sync.dma_start(
                xT_sb[:, :, 512 * n:512 * (n + 1)],
                xT[:, 512 * n:512 * (n + 1)]
                .rearrange("(a p) s -> p a s", p=128))

        dma_xT(0)
        nc.sync.dma_start(Wdkv_sb[:],
                          Wdkv.rearrange("(a p) l -> p a l", p=128))
        dma_xT(1)
        nc.sync.dma_start(Wukv_sb[:], Wukv[:])
        dma_xT(2)
        dma_xT(3)
        nc.sync.dma_start(Wq_sb[:, :, 0:128],
                          Wq[:, 0:128].rearrange("(a p) s -> p a s", p=128))
        tw_sb = sm.tile([32, 128], bf16, tag="tw")
        nc.sync.dma_start(tw_sb[:], Twedge[:])
        i32_sb = sm.tile([32, 32], bf16, tag="i32")
        nc.sync.dma_start(i32_sb[:], I32[:])
        ones_sb = sm.tile([1, 130], f32r, tag="ones")
        nc.sync.dma_start(ones_sb[:], Ones[:])
        bo_sb = sm.tile([1, D], f32r, tag="bo")
        nc.sync.dma_start(bo_sb[:], bo[:])
        nc.sync.dma_start(Wq_sb[:, :, 128:256],
                          Wq[:, 128:256].rearrange("(a p) s -> p a s", p=128))
        nc.sync.dma_start(Wq_sb[:, :, 256:D],
                          Wq[:, 256:D].rearrange("(a p) s -> p a s", p=128))
        nc.sync.dma_start(Wo_sb[:], Wo.rearrange("(a p) n -> p a n", p=128))

        latT_sb = big.tile([128, S], bf16, tag="latT")
        qT_sb = big.tile([128, H // 2, GQ], bf16, tag="qT")
        kT_sb = big.tile([128, H // 2, S], bf16, tag="kT")
        va_sb = big.tile([128, NKT, H * AUG], bf16, tag="va")
        ctxT_sb = big.tile([128, H // 2, GQ], bf16, tag="ctxT")
        bob_sb = sm.tile([128, D], f32r, tag="bob")

        # queries = columns 0::4 of (host-rotated) xT
        xq = xT_sb.rearrange("p a (q four) -> p a four q", four=4)

        # ones column of v_aug (disjoint from the va copies; no dep)
        nc.any.memset(
            va_sb[:].rearrange("p u (h e) -> p u h e", e=AUG)[:, :, :, HD],
            1.0)

        # GPSIMD cannot access PSUM, so all psum->sbuf copies go to DVE;
        # ScalarE is kept free for the exp stream.
        def copy(dst, src):
            nc.vector.tensor_copy(dst, src)

        # --------- background GEMM units (pout psums, [128,512]) -------
        def u_latT(n):
            def emit():
                ps = pout.tile([128, 512], f32, tag="p1", name=f"lat{n}")
                for k in range(D // 128):
                    nc.tensor.matmul(ps[:], Wdkv_sb[:, k, :],
                                     xT_sb[:, k, 512 * n:512 * (n + 1)],
                                     start=(k == 0), stop=(k == D // 128 - 1))
                copy(latT_sb[:, 512 * n:512 * (n + 1)], ps[:])
            return emit

        def u_qT(m):
            def emit():
                ps = pout.tile([128, 512], f32, tag="p1", name=f"q{m}")
                for k in range(D // 128):
                    nc.tensor.matmul(ps[:],
                                     Wq_sb[:, k, 128 * m:128 * (m + 1)],
                                     xq[:, k, 0, :],
                                     start=(k == 0), stop=(k == D // 128 - 1))
                copy(qT_sb[:, m, :], ps[:])
            return emit

        def u_kT(m, n):
            def emit():
                ps = pout.tile([128, 512], f32, tag="p1", name=f"k{m}_{n}")
                nc.tensor.matmul(ps[:], Wukv_sb[:, 128 * m:128 * (m + 1)],
                                 latT_sb[:, 512 * n:512 * (n + 1)],
                                 start=True, stop=True)
                copy(kT_sb[:, m, 512 * n:512 * (n + 1)], ps[:])
            return emit

        def u_va(u, half):
            def emit():
                ps = pout.tile([128, 512], f32, tag="p1",
                               name=f"v{u}_{half}")
                nc.tensor.matmul(
                    ps[:], latT_sb[:, 128 * u:128 * (u + 1)],
                    Wukv_sb[:, D + 512 * half:D + 512 * (half + 1)],
                    start=True, stop=True)
                dst = va_sb[:, u, AUG * 8 * half:AUG * 8 * (half + 1)]
                copy(dst.rearrange("p (h e) -> p h e", e=AUG)[:, :, 0:HD],
                     ps[:].rearrange("p (h e) -> p h e", e=HD))
            return emit

        def u_bob():
            for hh in range(2):
                ps = pout.tile([128, 512], f32, tag="p1", name=f"bob{hh}")
                nc.tensor.matmul(ps[:], ones_sb[0:1, 0:128],
                                 bo_sb[0:1, 512 * hh:512 * (hh + 1)],
                                 start=True, stop=True)
                copy(bob_sb[:, 512 * hh:512 * (hh + 1)], ps[:])

        # ---------------- attention stream -----------------------------
        def emit_scores(hp, bi):
            bitems, fill = bins[bi]
            sps = psc.tile([128, 1024], f32, tag="sc", name=f"s{hp}_{bi}")
            for par in range(2):
                p0 = 64 * par
                o0 = 512 * par
                for (u, cs, wedge), o in bitems:
                    sw = QT - cs
                    nc.tensor.matmul(
                        sps[:, o0 + o:o0 + o + sw],
                        kT_sb[p0:p0 + 64, hp, KT * u:KT * (u + 1)],
                        qT_sb[p0:p0 + 64, hp, cs:QT],
                        start=True, stop=not wedge)
                    if wedge:
                        wn = min(32, sw)
                        nc.tensor.matmul(sps[:, o0 + o:o0 + o + wn],
                                         tw_sb[:], i32_sb[:, 0:wn],
                                         start=False, stop=True)
            return sps

        def emit_exp(hp, bi, sps):
            bitems, fill = bins[bi]
            exps = sexp.tile([128, 1024], bf16, tag="exp",
                             name=f"e{hp}_{bi}")
            nc.scalar.activation(
                exps[:].rearrange("p (b c) -> p b c", b=2)[:, :, 0:fill],
                sps[:].rearrange("p (b c) -> p b c", b=2)[:, :, 0:fill],
                AF.Exp, scale=0.125)
            return exps

        cps_map = {}

        def emit_ctx(hp, bi, exps):
            bitems, fill = bins[bi]
            if hp not in cps_map:
                cps_map[hp] = ([pctx.tile([AUG, QT], f32, tag="ctx",
                                          name=f"c{hp}_{p}")
                                for p in range(2)], [0])
            cps, cnt = cps_map[hp]
            n_tot = len(items)
            for (u, cs, wedge), o in bitems:
                cnt[0] += 1
                for par in range(2):
                    h = hp * 2 + par
                    nc.tensor.matmul(
                        cps[par][:, cs:QT],
                        va_sb[:, u, AUG * h:AUG * (h + 1)],
                        exps[:, 512 * par + o:512 * par + o + (QT - cs)],
                        start=(cnt[0] == 1), stop=(cnt[0] == n_tot),
                        skip_group_check=True)
            if bi == NB - 1:
                _finish(hp, cps)
                del cps_map[hp]

        def _finish(hp, cps):
            # free cps fast: recip + parity copies first, then normalize
            recs = precs.tile([65, 2 * QT], f32r, tag="recs",
                              name=f"recs{hp}")
            recs0 = precs.tile([1, 2 * QT], f32r, tag="recs0",
                               name=f"recs0_{hp}")
            for par in range(2):
                rc = slice(par * QT, (par + 1) * QT)
                with nc.allow_low_precision(
                        reason="f32r is a bit-identical f32 alias"):
                    nc.vector.reciprocal(recs[64:65, rc],
                                         cps[par][HD:HD + 1, :])
            nc.vector.tensor_copy(ctxT_sb[0:64, hp, :], cps[0][0:HD, :])
            st = sout.tile([64, GQ], bf16, tag="st")
            nc.vector.tensor_copy(st[:], cps[1][0:HD, :])
            nc.sync.dma_start(ctxT_sb[64:128, hp, :], st[:])
            nc.sync.dma_start(recs0[:], recs[64:65, :])
            rb = sout.tile([128, GQ], f32r, tag="rb")
            for par in range(2):
                rp = pout.tile([64, GQ], f32, tag="p1", name=f"rp{hp}_{par}")
                nc.tensor.matmul(rp[:], ones_sb[0:1, 0:64],
                                 recs0[0:1, par * GQ:(par + 1) * GQ],
                                 start=True, stop=True)
                if par == 0:
                    nc.vector.tensor_copy(rb[0:64, :], rp[:])
                else:
                    st2 = sout.tile([64, GQ], f32r, tag="st2")
                    nc.vector.tensor_copy(st2[:], rp[:])
                    nc.sync.dma_start(rb[64:128, :], st2[:])
            nc.vector.tensor_tensor(ctxT_sb[:, hp, :], ctxT_sb[:, hp, :],
                                    rb[:], ALU.mult)

        # ------------- emission schedule -------------
        # prologue: latT chunks as xT lands, kT/va/qT for head-pair 0
        for n in range(4):
            u_latT(n)()
        for n in range(4):
            u_kT(0, n)()
        for u in range(NKT):
            u_va(u, 0)()
        u_qT(0)()
        u_bob()

        # background units due during attention: (flat_bin_index, emit_fn)
        bg = []
        for j in range(1, 8):
            base = (j - 1) * NB
            bg.append((base + 2, u_qT(j)))
            for n in range(4):
                bg.append((base + 5 + n, u_kT(j, n)))
        for k in range(NKT):
            bg.append((8 + 2 * k, u_va(k, 1)))
        bg.sort(key=lambda t: t[0])

        flat = [(hp, bi) for hp in range(H // 2) for bi in range(NB)]
        bgi = [0]
        pipe_sps = {}
        pipe_exps = {}

        def bg_drain(i):
            while bgi[0] < len(bg) and bg[bgi[0]][0] <= i:
                bg[bgi[0]][1]()
                bgi[0] += 1

        # per step i: scores(i+1) | exp(i) | bg GEMMs | ctx(i-1) — the bg
        # units sit between scores and ctx on the PE stream so the PE has
        # work while ScalarE finishes exp(i-1).
        pipe_sps[0] = emit_scores(*flat[0])
        for i in range(len(flat)):
            if i + 1 < len(flat):
                pipe_sps[i + 1] = emit_scores(*flat[i + 1])
            pipe_exps[i] = emit_exp(*flat[i], pipe_sps.pop(i))
            bg_drain(i)
            if i - 1 >= 0:
                emit_ctx(*flat[i - 1], pipe_exps.pop(i - 1))
        last = len(flat) - 1
        emit_ctx(*flat[last], pipe_exps.pop(last))

        # ---------------- output projection + bias ----------------
        for m in range(GQ // 128):
            for n in range(D // 512):
                ps = pout.tile([128, 512], f32, tag="p1", name=f"o{m}_{n}")
                for k in range(D // 128):
                    nc.tensor.matmul(
                        ps[:], ctxT_sb[:, k, 128 * m:128 * (m + 1)],
                        Wo_sb[:, k, 512 * n:512 * (n + 1)],
                        start=(k == 0), stop=(k == D // 128 - 1))
                ob = sout.tile([128, 512], f32, tag="ob")
                nc.vector.tensor_tensor(
                    ob[:], ps[:], bob_sb[:, 512 * n:512 * (n + 1)],
                    ALU.add)
                nc.sync.dma_start(
                    out[128 * m:128 * (m + 1), 512 * n:512 * (n + 1)], ob[:])


def _in_maps(x, offset, Wq, Wdkv, Wukv, Wo, bo):
    import ml_dtypes
    items = _worklist(offset)
    f32 = np.float32
    bff = ml_dtypes.bfloat16
    maps = []
    i32 = np.eye(32, dtype=bff)
    common = {
        "Wq": np.ascontiguousarray(Wq).astype(bff),
        "Wdkv": np.ascontiguousarray(Wdkv).astype(bff),
        "Wukv": np.ascontiguousarray(Wukv).astype(bff),
        "Wo": np.ascontiguousarray(Wo).astype(bff),
        "bo": np.ascontiguousarray(bo, f32).reshape(1, D),
        "I32": i32,
        "Ones": np.ones((1, 130), f32),
    }
    for c in range(NCORES):
        b, g = c // 4, c % 4
        m = dict(common)
        # rotate x columns so this core's queries are columns 0::4:
        # core-local column 4*t+r holds global row 4*t + ((r+g) % 4).
        perm = (np.arange(S) // 4) * 4 + (np.arange(S) + g) % 4
        m["xT"] = np.ascontiguousarray(x[b][perm].T).astype(bff)
        m["Twedge"] = _wedge_matrix(g, offset, items).astype(bff)
        maps.append(m)
    return maps


def kernel(x, offset, Wq, Wdkv, Wukv, Wo, bo):
    from concourse.bass_utils import run_bass_kernel_spmd
    off = int(np.asarray(offset))
    if off not in _cache:
        _cache[off] = _build(off)
    nc = _cache[off]
    maps = _in_maps(np.asarray(x, np.float32), off, Wq, Wdkv, Wukv, Wo, bo)
    res = run_bass_kernel_spmd(nc, maps, list(range(NCORES)))
    outf = np.empty((B, S, D), np.float32)
    for c in range(NCORES):
        b, g = c // 4, c % 4
        outf[b, g::4, :] = res.results[c]["out"]
    return outf


# revision 37
# speedup vs baseline: 2.9186x; 2.9186x over previous
"""Multi-Head Latent Attention (naive MLA) on 8 Trainium2 NeuronCores.

Sharding: data-parallel over batch (2) x causal-balanced sequence-parallel
over queries (4-way interleave): core c handles batch b = c//4, query group
g = c%4 (global query rows g, g+4, g+8, ...).  Every core runs the identical
SPMD program; only the data differs.  To keep the program data-independent,
the host rotates each x-column group of 4 so the core's queries sit at
columns 0::4 (keys are therefore mildly permuted within each group of 4;
the wedge-mask matrix, also host data, accounts for that permutation).
No collectives: each core produces the full output rows for its queries.

All matmuls contract over the SBUF partition dim, so everything is kept
"transposed" (feature-major) end to end and no on-device transposes are
needed.  Scores never materialize K: queries are projected into latent
space instead (k_h.q_h = (Wuk_h latent).q_h = latent.(Wuk_h^T q_h)):
  latentT = Wdkv^T @ x^T                  [128, 2048]  bf16
  qT      = Wq^T @ xT[:, 0::4]            [1024, 512]  bf16
  qL_h    = Wuk_h^T @ qT_h                [128, 512] per head, bf16
  v_aug   = [latent @ Wuv | ones] per key tile          bf16
  scoresT: per head pair, one 2-bank psum [128 keys, 2x512]: parity p's
           scores (latT_u^T @ qL) in columns 512p..; causal wedge added by
           a rank-32 mask matmul; ONE fused exp per bin on ScalarE covers
           both parities (1/sqrt(hd) folded into the activation scale);
           softmax denom comes for free as row 64 of the ctx matmul (ones
           column of v_aug)
  ctxT_h  = v_aug^T @ expT                [65, 512] psum accum over key tiles
  out     = matmul(lhsT=ctxT tiles, rhs=Wo); bias added by the psum->sbuf
            copy (tensor_tensor with a broadcast bias tile)  -> [512, 1024]

The attention stream is software-pipelined (scores one bin ahead of exp,
exp two bins ahead of ctx) and the phase-1/2 GEMMs (qT / qL / v_aug
chunks) plus the deferred per-head normalizations are emitted BETWEEN
attention bins so the PE keeps running while ScalarE does exp; psum->sbuf
copies run on DVE (GPSIMD cannot touch PSUM; it does the SBUF-only
normalization multiplies instead).
PSUM: psc 2x[128,1024] for scores (4 banks), pctx 2x[65,512] ctx accum
(2 banks), pout 2x[128,512] for background GEMMs / bias / out-proj.
"""

import numpy as np

B, S, D, L, H = 2, 2048, 1024, 128, 16
HD = D // H        # 64
AUG = HD + 1       # 65 (v dims + ones column for softmax denominator)
NCORES = 8
GQ = S // 4        # 512 queries per core
QT = 512           # queries per tile (single tile)
KT = 128           # keys per key tile
NKT = S // KT      # 16
NEG = -640.0       # additive mask pre-exp-scale (x 1/8 -> -80)

_cache = {}


def _worklist(offset):
    """Strip list [(u, cs, wedge)]; identical across cores.

    Query column c = global row 4*c+g, position +offset.  cs (first
    computed column of the strip) uses the worst core (g=3) so strip
    shapes are core-independent; the wedge matrix (data) carries g.
    """
    items = []
    for u in range(NKT):
        lo = KT * u
        min_qpos = 0 + offset
        max_qpos = 4 * (QT - 1) + 3 + offset
        if lo + KT - 1 <= min_qpos:
            items.append((u, 0, False))      # fully allowed
        elif lo > max_qpos:
            continue                         # fully masked: skip
        else:
            cs = max(0, -((-(lo - 3 - offset)) // 4))
            assert 0 <= cs < QT
            items.append((u, cs, True))
    assert items and items[0][1] == 0, "first strip must cover col 0"
    return items


def _wedge_matrix(g, offset, items):
    """[32, 128] f32: T[m, j] = NEG where local key j is masked at strip
    col m.  Host rotates x columns so queries are 0::4; local key index j
    of a strip is global position lo + pi(j), pi(j) = 4*(j//4)+(j%4+g)%4.
    Masked iff lo+pi(j) > q_pos = 4*(cs+m)+g+offset, i.e. pi(j) > 4*m+r0,
    r0 = 4*cs+g+offset-lo (strip-independent; asserted).
    """
    r0s = set()
    for (u, cs, wedge) in items:
        if wedge:
            r0s.add(4 * cs + g + offset - KT * u)
    if not r0s:
        r0s = {g}
    assert len(r0s) == 1, f"non-uniform wedge r0 {r0s} (offset={offset})"
    r0 = r0s.pop()
    assert 0 <= r0 <= 127, r0
    j = np.arange(128)
    pi = 4 * (j // 4) + (j % 4 + g) % 4
    T = np.zeros((32, 128), np.float32)
    for m in range(32):
        T[m, :] = np.where(pi > 4 * m + r0, NEG, 0.0)
    return T


def _blocks_of(items):
    """Pack strips into per-parity psum bins of <=512 cols (one matmul's
    psum output can't cross a bank).  Returns [(list[(item, off)], fill)]."""
    bins = []
    cur, w = [], 0
    for it in items:
        sw = QT - it[1]
        if w + sw > 512:
            bins.append((cur, w))
            cur, w = [], 0
        cur.append((it, w))
        w += sw
    if cur:
        bins.append((cur, w))
    return bins


def _build(offset, reps=1):
    import concourse.bacc as bacc
    import concourse.tile as tile
    import concourse.mybir as mybir
    from contextlib import ExitStack

    f32r = mybir.dt.float32r
    bf16 = mybir.dt.bfloat16
    f32 = mybir.dt.float32

    nc = bacc.Bacc("TRN2", target_bir_lowering=False, debug=False,
                   num_devices=NCORES)
    xT = nc.dram_tensor("xT", [D, S], bf16, kind="ExternalInput").ap()
    Wq = nc.dram_tensor("Wq", [D, D], bf16, kind="ExternalInput").ap()
    Wdkv = nc.dram_tensor("Wdkv", [D, L], bf16, kind="ExternalInput").ap()
    Wukv = nc.dram_tensor("Wukv", [L, 2 * D], bf16, kind="ExternalInput").ap()
    WukT = nc.dram_tensor("WukT", [D, L], bf16, kind="ExternalInput").ap()
    Wo = nc.dram_tensor("Wo", [D, D], bf16, kind="ExternalInput").ap()
    bo = nc.dram_tensor("bo", [1, D], f32r, kind="ExternalInput").ap()
    Twedge = nc.dram_tensor("Twedge", [32, 128], bf16,
                            kind="ExternalInput").ap()
    I32 = nc.dram_tensor("I32", [32, 32], bf16, kind="ExternalInput").ap()
    Ones = nc.dram_tensor("Ones", [1, 130], f32r, kind="ExternalInput").ap()
    out = nc.dram_tensor("out", [GQ, D], f32, kind="ExternalOutput").ap()

    for _rep in range(reps):
        _emit_body(nc, tile, mybir, ExitStack, offset,
                   xT, Wq, Wdkv, Wukv, WukT, Wo, bo, Twedge, I32, Ones, out)

    nc.compile()
    return nc


def _emit_body(nc, tile, mybir, ExitStack, offset,
               xT, Wq, Wdkv, Wukv, WukT, Wo, bo, Twedge, I32, Ones, out):
    f32r = mybir.dt.float32r
    bf16 = mybir.dt.bfloat16
    f32 = mybir.dt.float32
    AF = mybir.ActivationFunctionType
    ALU = mybir.AluOpType

    items = _worklist(offset)
    bins = _blocks_of(items)
    NB = len(bins)

    with tile.TileContext(nc) as tc, ExitStack() as ctx:
        big = ctx.enter_context(tc.tile_pool(name="big", bufs=1, side="left"))
        sm = ctx.enter_context(tc.tile_pool(name="sm", bufs=1, side="right"))
        sexp = ctx.enter_context(tc.tile_pool(name="sexp", bufs=4,
                                              side="right"))
        sout = ctx.enter_context(tc.tile_pool(name="sout", bufs=2,
                                              side="right"))
        precs = ctx.enter_context(tc.tile_pool(name="precs", bufs=2,
                                               side="right"))
        psc = ctx.enter_context(tc.tile_pool(name="psc", bufs=2,
                                             space="PSUM", side="left"))
        pctx = ctx.enter_context(tc.tile_pool(name="pctx", bufs=2,
                                              space="PSUM", side="right"))
        pout = ctx.enter_context(tc.tile_pool(name="pout", bufs=2,
                                              space="PSUM", side="right"))

        # ------------- input DMAs (dependency-priority order) -------------
        xT_sb = big.tile([128, D // 128, S], bf16, tag="xT")
        Wdkv_sb = big.tile([128, D // 128, L], bf16, tag="Wdkv")
        Wukv_sb = big.tile([128, D], bf16, tag="Wuv")
        Wq_sb = big.tile([128, D // 128, D], bf16, tag="Wq")
        Wo_sb = big.tile([128, D // 128, D], bf16, tag="Wo")

        def dma_xT(n):
            nc.sync.dma_start(
                xT_sb[:, :, 512 * n:512 * (n + 1)],
                xT[:, 512 * n:512 * (n + 1)]
                .rearrange("(a p) s -> p a s", p=128))

        dma_xT(0)
        nc.sync.dma_start(Wdkv_sb[:],
                          Wdkv.rearrange("(a p) l -> p a l", p=128))
        dma_xT(1)
        nc.sync.dma_start(Wukv_sb[:], Wukv[:, D:2 * D])
        dma_xT(2)
        dma_xT(3)
        nc.sync.dma_start(Wq_sb[:, :, 0:128],
                          Wq[:, 0:128].rearrange("(a p) s -> p a s", p=128))
        WukT_sb = big.tile([128, D // 128, L], bf16, tag="WukT")
        nc.sync.dma_start(WukT_sb[:],
                          WukT.rearrange("(a p) l -> p a l", p=128))
        tw_sb = sm.tile([32, 128], bf16, tag="tw")
        nc.sync.dma_start(tw_sb[:], Twedge[:])
        i32_sb = sm.tile([32, 32], bf16, tag="i32")
        nc.sync.dma_start(i32_sb[:], I32[:])
        ones_sb = sm.tile([1, 130], f32r, tag="ones")
        nc.sync.dma_start(ones_sb[:], Ones[:])
        bo_sb = sm.tile([1, D], f32r, tag="bo")
        nc.sync.dma_start(bo_sb[:], bo[:])
        nc.sync.dma_start(Wq_sb[:, :, 128:256],
                          Wq[:, 128:256].rearrange("(a p) s -> p a s", p=128))
        nc.sync.dma_start(Wq_sb[:, :, 256:D],
                          Wq[:, 256:D].rearrange("(a p) s -> p a s", p=128))
        nc.sync.dma_start(Wo_sb[:], Wo.rearrange("(a p) n -> p a n", p=128))

        latT_sb = big.tile([128, S], bf16, tag="latT")
        qT_sb = big.tile([128, H // 2, GQ], bf16, tag="qT")
        qL_sb = big.tile([128, H, GQ], bf16, tag="qL")
        va_sb = big.tile([128, NKT, H * AUG], bf16, tag="va")
        ctxT_sb = big.tile([128, H // 2, GQ], bf16, tag="ctxT")
        bob_sb = sm.tile([128, D], f32r, tag="bob")

        # queries = columns 0::4 of (host-rotated) xT
        xq = xT_sb.rearrange("p a (q four) -> p a four q", four=4)

        # ones column of v_aug (disjoint from the va copies; no dep)
        nc.any.memset(
            va_sb[:].rearrange("p u (h e) -> p u h e", e=AUG)[:, :, :, HD],
            1.0)

        # GPSIMD cannot access PSUM.  Prologue psum->sbuf copies alternate
        # DVE / ScalarE (exp hasn't started); in-attention copies go to DVE
        # so ScalarE is kept free for the exp stream.
        state = {"attn": False, "flip": 0}

        def copy(dst, src):
            if state["attn"]:
                nc.vector.tensor_copy(dst, src)
            elif state["flip"] % 2 == 0:
                nc.vector.tensor_copy(dst, src)
            else:
                nc.scalar.copy(dst, src)
            state["flip"] += 1

        # --------- background GEMM units (pout psums, [128,512]) -------
        def u_latT(n):
            def emit():
                ps = pout.tile([128, 512], f32, tag="p1", name=f"lat{n}")
                for k in range(D // 128):
                    nc.tensor.matmul(ps[:], Wdkv_sb[:, k, :],
                                     xT_sb[:, k, 512 * n:512 * (n + 1)],
                                     start=(k == 0), stop=(k == D // 128 - 1))
                copy(latT_sb[:, 512 * n:512 * (n + 1)], ps[:])
            return emit

        def u_qT(m):
            def emit():
                ps = pout.tile([128, 512], f32, tag="p1", name=f"q{m}")
                for k in range(D // 128):
                    nc.tensor.matmul(ps[:],
                                     Wq_sb[:, k, 128 * m:128 * (m + 1)],
                                     xq[:, k, 0, :],
                                     start=(k == 0), stop=(k == D // 128 - 1))
                copy(qT_sb[:, m, :], ps[:])
            return emit

        def u_qL(h):
            # project head h's queries into latent space: qL_h = Wuk_h^T q_h
            def emit():
                ps = pout.tile([128, 512], f32, tag="p1", name=f"ql{h}")
                p0 = 64 * (h % 2)
                nc.tensor.matmul(ps[:],
                                 WukT_sb[p0:p0 + 64, h // 2, :],
                                 qT_sb[p0:p0 + 64, h // 2, :],
                                 start=True, stop=True)
                copy(qL_sb[:, h, :], ps[:])
            return emit

        def u_va(u, half):
            def emit():
                ps = pout.tile([128, 512], f32, tag="p1",
                               name=f"v{u}_{half}")
                nc.tensor.matmul(
                    ps[:], latT_sb[:, 128 * u:128 * (u + 1)],
                    Wukv_sb[:, 512 * half:512 * (half + 1)],
                    start=True, stop=True)
                dst = va_sb[:, u, AUG * 8 * half:AUG * 8 * (half + 1)]
                copy(dst.rearrange("p (h e) -> p h e", e=AUG)[:, :, 0:HD],
                     ps[:].rearrange("p (h e) -> p h e", e=HD))
            return emit

        def u_bob():
            for hh in range(2):
                ps = pout.tile([128, 512], f32, tag="p1", name=f"bob{hh}")
                nc.tensor.matmul(ps[:], ones_sb[0:1, 0:128],
                                 bo_sb[0:1, 512 * hh:512 * (hh + 1)],
                                 start=True, stop=True)
                copy(bob_sb[:, 512 * hh:512 * (hh + 1)], ps[:])

        # ---------------- attention stream -----------------------------
        def emit_scores(hp, bi):
            bitems, fill = bins[bi]
            sps = psc.tile([128, 1024], f32, tag="sc", name=f"s{hp}_{bi}")
            for par in range(2):
                o0 = 512 * par
                h = 2 * hp + par
                for (u, cs, wedge), o in bitems:
                    sw = QT - cs
                    nc.tensor.matmul(
                        sps[:, o0 + o:o0 + o + sw],
                        latT_sb[:, KT * u:KT * (u + 1)],
                        qL_sb[:, h, cs:QT],
                        start=True, stop=not wedge)
                    if wedge:
                        wn = min(32, sw)
                        nc.tensor.matmul(sps[:, o0 + o:o0 + o + wn],
                                         tw_sb[:], i32_sb[:, 0:wn],
                                         start=False, stop=True)
            return sps

        def emit_exp(hp, bi, sps):
            bitems, fill = bins[bi]
            exps = sexp.tile([128, 1024], bf16, tag="exp",
                             name=f"e{hp}_{bi}")
            nc.scalar.activation(
                exps[:].rearrange("p (b c) -> p b c", b=2)[:, :, 0:fill],
                sps[:].rearrange("p (b c) -> p b c", b=2)[:, :, 0:fill],
                AF.Exp, scale=0.125)
            return exps

        cps_map = {}

        def emit_ctx(hp, bi, exps):
            bitems, fill = bins[bi]
            if hp not in cps_map:
                cps_map[hp] = ([pctx.tile([AUG, QT], f32, tag="ctx",
                                          name=f"c{hp}_{p}")
                                for p in range(2)], [0])
            cps, cnt = cps_map[hp]
            n_tot = len(items)
            for (u, cs, wedge), o in bitems:
                cnt[0] += 1
                for par in range(2):
                    h = hp * 2 + par
                    nc.tensor.matmul(
                        cps[par][:, cs:QT],
                        va_sb[:, u, AUG * h:AUG * (h + 1)],
                        exps[:, 512 * par + o:512 * par + o + (QT - cs)],
                        start=(cnt[0] == 1), stop=(cnt[0] == n_tot),
                        skip_group_check=True)
            if bi == NB - 1:
                _finish_a(hp, cps)
                del cps_map[hp]

        recs0_map = {}

        def _finish_a(hp, cps):
            # free cps fast: recip + parity copies; the reciprocal row is
            # DMA'd to partition 0 now so the broadcast matmuls (_finish_b,
            # emitted a few bins later) never stall the PE on the DMA.
            recs = precs.tile([65, 2 * QT], f32r, tag="recs",
                              name=f"recs{hp}")
            recs0 = precs.tile([1, 2 * QT], f32r, tag="recs0",
                               name=f"recs0_{hp}")
            for par in range(2):
                rc = slice(par * QT, (par + 1) * QT)
                with nc.allow_low_precision(
                        reason="f32r is a bit-identical f32 alias"):
                    nc.vector.reciprocal(recs[64:65, rc],
                                         cps[par][HD:HD + 1, :])
            nc.vector.tensor_copy(ctxT_sb[0:64, hp, :], cps[0][0:HD, :])
            st = sout.tile([64, GQ], bf16, tag="st")
            nc.vector.tensor_copy(st[:], cps[1][0:HD, :])
            nc.sync.dma_start(ctxT_sb[64:128, hp, :], st[:])
            nc.sync.dma_start(recs0[:], recs[64:65, :])
            recs0_map[hp] = recs0

        def _finish_b(hp):
            recs0 = recs0_map.pop(hp)
            rb = sout.tile([128, GQ], f32r, tag="rb")
            for par in range(2):
                rp = pout.tile([64, GQ], f32, tag="p1", name=f"rp{hp}_{par}")
                nc.tensor.matmul(rp[:], ones_sb[0:1, 0:64],
                                 recs0[0:1, par * GQ:(par + 1) * GQ],
                                 start=True, stop=True)
                if par == 0:
                    nc.vector.tensor_copy(rb[0:64, :], rp[:])
                else:
                    st2 = sout.tile([64, GQ], f32r, tag="st2")
                    nc.vector.tensor_copy(st2[:], rp[:])
                    nc.sync.dma_start(rb[64:128, :], st2[:])
            # SBUF-only, so it can run on the otherwise-idle GPSIMD engine
            nc.gpsimd.tensor_tensor(ctxT_sb[:, hp, :], ctxT_sb[:, hp, :],
                                    rb[:], ALU.mult)

        # ------------- emission schedule -------------
        # Prologue uses the (still idle) 2-bank psc tiles so two 512-col
        # chunks share one psum tile and one copy: fewer, larger copies and
        # a 4-slot psum rotation instead of 2.
        def pro_latT(h):
            ps = psc.tile([128, 1024], f32, tag="sc", name=f"plat{h}")
            for nn in range(2):
                n = 2 * h + nn
                for k in range(D // 128):
                    nc.tensor.matmul(ps[:, 512 * nn:512 * (nn + 1)],
                                     Wdkv_sb[:, k, :],
                                     xT_sb[:, k, 512 * n:512 * (n + 1)],
                                     start=(k == 0), stop=(k == D // 128 - 1))
            copy(latT_sb[:, 1024 * h:1024 * (h + 1)], ps[:])

        def pro_qL01():
            ps = psc.tile([128, 1024], f32, tag="sc", name="pql")
            for h in range(2):
                p0 = 64 * h
                nc.tensor.matmul(ps[:, 512 * h:512 * (h + 1)],
                                 WukT_sb[p0:p0 + 64, 0, :],
                                 qT_sb[p0:p0 + 64, 0, :],
                                 start=True, stop=True)
            copy(qL_sb[:, 0:2, :],
                 ps[:].rearrange("p (h q) -> p h q", h=2))

        def pro_va(p):
            ps = psc.tile([128, 1024], f32, tag="sc", name=f"pv{p}")
            for uu in range(2):
                u = 2 * p + uu
                nc.tensor.matmul(
                    ps[:, 512 * uu:512 * (uu + 1)],
                    latT_sb[:, 128 * u:128 * (u + 1)],
                    Wukv_sb[:, 0:512],
                    start=True, stop=True)
            dst = va_sb[:, 2 * p:2 * p + 2, 0:AUG * 8]
            copy(dst.rearrange("p u (h e) -> p u h e", e=AUG)[:, :, :, 0:HD],
                 ps[:].rearrange("p (u h e) -> p u h e", u=2, e=HD))

        pro_latT(0)
        pro_latT(1)
        u_qT(0)()
        pro_qL01()
        for p in range(NKT // 2):
            pro_va(p)
        u_bob()
        state["attn"] = True

        # background units due during attention: (flat_bin_index, emit_fn)
        bg = []
        for j in range(1, 8):
            base = (j - 1) * NB
            bg.append((base + 2, u_qT(j)))
            bg.append((base + 5, u_qL(2 * j)))
            bg.append((base + 6, u_qL(2 * j + 1)))
        for k in range(NKT):
            bg.append((8 + 2 * k, u_va(k, 1)))
        for j in range(7):
            # normalization of head-pair j, a few bins into head-pair j+1
            bg.append((j * NB + NB + 4, lambda j=j: _finish_b(j)))
        bg.sort(key=lambda t: t[0])

        flat = [(hp, bi) for hp in range(H // 2) for bi in range(NB)]
        bgi = [0]
        pipe_sps = {}
        pipe_exps = {}

        def bg_drain(i):
            while bgi[0] < len(bg) and bg[bgi[0]][0] <= i:
                bg[bgi[0]][1]()
                bgi[0] += 1

        # per step i: scores(i+1) | exp(i) | bg GEMMs | ctx(i-2) — the bg
        # units sit between scores and ctx on the PE stream so the PE has
        # work while ScalarE finishes exp; ctx lags exp by two bins so the
        # PE never waits on a just-issued activation.
        pipe_sps[0] = emit_scores(*flat[0])
        for i in range(len(flat)):
            if i + 1 < len(flat):
                pipe_sps[i + 1] = emit_scores(*flat[i + 1])
            pipe_exps[i] = emit_exp(*flat[i], pipe_sps.pop(i))
            bg_drain(i)
            if i - 2 >= 0:
                emit_ctx(*flat[i - 2], pipe_exps.pop(i - 2))
        for i in (len(flat) - 2, len(flat) - 1):
            emit_ctx(*flat[i], pipe_exps.pop(i))
        _finish_b(7)

        # ---------------- output projection + bias ----------------
        for m in range(GQ // 128):
            for n in range(D // 512):
                ps = pout.tile([128, 512], f32, tag="p1", name=f"o{m}_{n}")
                for k in range(D // 128):
                    nc.tensor.matmul(
                        ps[:], ctxT_sb[:, k, 128 * m:128 * (m + 1)],
                        Wo_sb[:, k, 512 * n:512 * (n + 1)],
                        start=(k == 0), stop=(k == D // 128 - 1))
                ob = sout.tile([128, 512], f32, tag="ob")
                nc.vector.tensor_tensor(
                    ob[:], ps[:], bob_sb[:, 512 * n:512 * (n + 1)],
                    ALU.add)
                nc.sync.dma_start(
                    out[128 * m:128 * (m + 1), 512 * n:512 * (n + 1)], ob[:])


def _in_maps(x, offset, Wq, Wdkv, Wukv, Wo, bo):
    import ml_dtypes
    items = _worklist(offset)
    f32 = np.float32
    bff = ml_dtypes.bfloat16
    maps = []
    i32 = np.eye(32, dtype=bff)
    common = {
        "Wq": np.ascontiguousarray(Wq).astype(bff),
        "Wdkv": np.ascontiguousarray(Wdkv).astype(bff),
        "Wukv": np.ascontiguousarray(Wukv).astype(bff),
        "WukT": np.ascontiguousarray(np.asarray(Wukv)[:, :D].T).astype(bff),
        "Wo": np.ascontiguousarray(Wo).astype(bff),
        "bo": np.ascontiguousarray(bo, f32).reshape(1, D),
        "I32": i32,
        "Ones": np.ones((1, 130), f32),
    }
    for c in range(NCORES):
        b, g = c // 4, c % 4
        m = dict(common)
        # rotate x columns so this core's queries are columns 0::4:
        # core-local column 4*t+r holds global row 4*t + ((r+g) % 4).
        perm = (np.arange(S) // 4) * 4 + (np.arange(S) + g) % 4
        m["xT"] = np.ascontiguousarray(x[b][perm].T).astype(bff)
        m["Twedge"] = _wedge_matrix(g, offset, items).astype(bff)
        maps.append(m)
    return maps


def kernel(x, offset, Wq, Wdkv, Wukv, Wo, bo):
    from concourse.bass_utils import run_bass_kernel_spmd
    off = int(np.asarray(offset))
    if off not in _cache:
        _cache[off] = _build(off)
    nc = _cache[off]
    maps = _in_maps(np.asarray(x, np.float32), off, Wq, Wdkv, Wukv, Wo, bo)
    res = run_bass_kernel_spmd(nc, maps, list(range(NCORES)))
    outf = np.empty((B, S, D), np.float32)
    for c in range(NCORES):
        b, g = c // 4, c % 4
        outf[b, g::4, :] = res.results[c]["out"]
    return outf


# revision 44
# speedup vs baseline: 4.1348x; 1.4167x over previous
"""Multi-Head Latent Attention (naive MLA) on 8 Trainium2 NeuronCores.

Sharding: data-parallel over batch (2) x causal-balanced sequence-parallel
over queries (4-way interleave): core c handles batch b = c//4, query group
g = c%4 (global query rows g, g+4, g+8, ...).  Every core runs the identical
SPMD program; only the data differs.  To keep the program data-independent,
the host rotates each x-column group of 4 so the core's queries sit at
columns 0::4 (keys are therefore mildly permuted within each group of 4;
the wedge-mask matrix, also host data, accounts for that permutation).
No collectives: each core produces the full output rows for its queries.

All matmuls contract over the SBUF partition dim, so everything is kept
"transposed" (feature-major) end to end and no on-device transposes are
needed.  Scores never materialize K: queries are projected into latent
space instead (k_h.q_h = (Wuk_h latent).q_h = latent.(Wuk_h^T q_h)):
  latentT = Wdkv^T @ x^T                  [128, 2048]  bf16
  qT      = Wq^T @ xT[:, 0::4]            [1024, 512]  bf16
  qL_h    = Wuk_h^T @ qT_h                [128, 512] per head, bf16
  v_aug   = [latent @ Wuv | ones] per key tile          bf16
  scoresT: per head pair, one 2-bank psum [128 keys, 2x512]: parity p's
           scores (latT_u^T @ qL) in columns 512p..; causal wedge added by
           a rank-32 mask matmul; ONE fused exp per bin on ScalarE covers
           both parities (1/sqrt(hd) folded into the activation scale);
           softmax denom comes for free as row 64 of the ctx matmul (ones
           column of v_aug)
  ctxT_h  = v_aug^T @ expT                [65, 512] psum accum over key tiles
  out     = matmul(lhsT=ctxT tiles, rhs=Wo); bias added by the psum->sbuf
            copy (tensor_tensor with a broadcast bias tile)  -> [512, 1024]

The attention stream is software-pipelined (scores one bin ahead of exp,
exp two bins ahead of ctx) and the phase-1/2 GEMMs (qT / qL / v_aug
chunks) plus the deferred per-head normalizations are emitted BETWEEN
attention bins so the PE keeps running while ScalarE does exp; psum->sbuf
copies run on DVE (GPSIMD cannot touch PSUM; it does the SBUF-only
normalization multiplies instead).
PSUM: psc 2x[128,1024] for scores (4 banks), pctx 2x[65,512] ctx accum
(2 banks), pout 2x[128,512] for background GEMMs / bias / out-proj.
"""

import numpy as np

B, S, D, L, H = 2, 2048, 1024, 128, 16
HD = D // H        # 64
AUG = HD + 1       # 65 (v dims + ones column for softmax denominator)
NCORES = 8
GQ = S // 4        # 512 queries per core
QT = 512           # queries per tile (single tile)
KT = 128           # keys per key tile
NKT = S // KT      # 16
NEG = -640.0       # additive mask pre-exp-scale (x 1/8 -> -80)

_cache = {}


def _worklist(offset):
    """Strip list [(u, cs, wedge)]; identical across cores.

    Query column c = global row 4*c+g, position +offset.  cs (first
    computed column of the strip) uses the worst core (g=3) so strip
    shapes are core-independent; the wedge matrix (data) carries g.
    """
    items = []
    for u in range(NKT):
        lo = KT * u
        min_qpos = 0 + offset
        max_qpos = 4 * (QT - 1) + 3 + offset
        if lo + KT - 1 <= min_qpos:
            items.append((u, 0, False))      # fully allowed
        elif lo > max_qpos:
            continue                         # fully masked: skip
        else:
            cs = max(0, -((-(lo - 3 - offset)) // 4))
            assert 0 <= cs < QT
            items.append((u, cs, True))
    assert items and items[0][1] == 0, "first strip must cover col 0"
    return items


def _wedge_matrix(g, offset, items):
    """[32, 128] f32: T[m, j] = NEG where local key j is masked at strip
    col m.  Host rotates x columns so queries are 0::4; local key index j
    of a strip is global position lo + pi(j), pi(j) = 4*(j//4)+(j%4+g)%4.
    Masked iff lo+pi(j) > q_pos = 4*(cs+m)+g+offset, i.e. pi(j) > 4*m+r0,
    r0 = 4*cs+g+offset-lo (strip-independent; asserted).
    """
    r0s = set()
    for (u, cs, wedge) in items:
        if wedge:
            r0s.add(4 * cs + g + offset - KT * u)
    if not r0s:
        r0s = {g}
    assert len(r0s) == 1, f"non-uniform wedge r0 {r0s} (offset={offset})"
    r0 = r0s.pop()
    assert 0 <= r0 <= 127, r0
    j = np.arange(128)
    pi = 4 * (j // 4) + (j % 4 + g) % 4
    T = np.zeros((32, 128), np.float32)
    for m in range(32):
        T[m, :] = np.where(pi > 4 * m + r0, NEG, 0.0)
    return T


def _blocks_of(items):
    """Pack strips into per-parity psum bins of <=512 cols (one matmul's
    psum output can't cross a bank).  Returns [(list[(item, off)], fill)]."""
    bins = []
    cur, w = [], 0
    for it in items:
        sw = QT - it[1]
        if w + sw > 512:
            bins.append((cur, w))
            cur, w = [], 0
        cur.append((it, w))
        w += sw
    if cur:
        bins.append((cur, w))
    return bins


def _build(offset, reps=1):
    import concourse.bacc as bacc
    import concourse.tile as tile
    import concourse.mybir as mybir
    from contextlib import ExitStack

    f32r = mybir.dt.float32r
    bf16 = mybir.dt.bfloat16
    f32 = mybir.dt.float32

    nc = bacc.Bacc("TRN2", target_bir_lowering=False, debug=False,
                   num_devices=NCORES)
    xT = nc.dram_tensor("xT", [D, S], bf16, kind="ExternalInput").ap()
    Wq = nc.dram_tensor("Wq", [D, D], bf16, kind="ExternalInput").ap()
    Wdkv = nc.dram_tensor("Wdkv", [D, L], bf16, kind="ExternalInput").ap()
    Wukv = nc.dram_tensor("Wukv", [L, 2 * D], bf16, kind="ExternalInput").ap()
    WukT = nc.dram_tensor("WukT", [D, L], bf16, kind="ExternalInput").ap()
    Wo = nc.dram_tensor("Wo", [D, D], bf16, kind="ExternalInput").ap()
    bo = nc.dram_tensor("bo", [1, D], f32r, kind="ExternalInput").ap()
    Twedge = nc.dram_tensor("Twedge", [32, 128], bf16,
                            kind="ExternalInput").ap()
    I32 = nc.dram_tensor("I32", [32, 32], bf16, kind="ExternalInput").ap()
    E2 = nc.dram_tensor("E2", [2, 128], f32r, kind="ExternalInput").ap()
    Ones = nc.dram_tensor("Ones", [1, 130], f32r, kind="ExternalInput").ap()
    out = nc.dram_tensor("out", [GQ, D], f32, kind="ExternalOutput").ap()

    for _rep in range(reps):
        _emit_body(nc, tile, mybir, ExitStack, offset,
                   xT, Wq, Wdkv, Wukv, WukT, Wo, bo, Twedge, I32, E2, Ones,
                   out)

    nc.compile()
    return nc


def _emit_body(nc, tile, mybir, ExitStack, offset,
               xT, Wq, Wdkv, Wukv, WukT, Wo, bo, Twedge, I32, E2, Ones, out):
    f32r = mybir.dt.float32r
    bf16 = mybir.dt.bfloat16
    f32 = mybir.dt.float32
    AF = mybir.ActivationFunctionType
    ALU = mybir.AluOpType

    items = _worklist(offset)
    bins = _blocks_of(items)
    NB = len(bins)

    with tile.TileContext(nc) as tc, ExitStack() as ctx:
        big = ctx.enter_context(tc.tile_pool(name="big", bufs=1, side="left"))
        sm = ctx.enter_context(tc.tile_pool(name="sm", bufs=1, side="right"))
        sexp = ctx.enter_context(tc.tile_pool(name="sexp", bufs=4,
                                              side="right"))
        sout = ctx.enter_context(tc.tile_pool(name="sout", bufs=2,
                                              side="right"))
        precs = ctx.enter_context(tc.tile_pool(name="precs", bufs=2,
                                               side="right"))
        psc = ctx.enter_context(tc.tile_pool(name="psc", bufs=2,
                                             space="PSUM", side="left"))
        pctx = ctx.enter_context(tc.tile_pool(name="pctx", bufs=2,
                                              space="PSUM", side="right"))
        pout = ctx.enter_context(tc.tile_pool(name="pout", bufs=2,
                                              space="PSUM", side="right"))

        # ------------- input DMAs (dependency-priority order) -------------
        xT_sb = big.tile([128, D // 128, S], bf16, tag="xT")
        Wdkv_sb = big.tile([128, D // 128, L], bf16, tag="Wdkv")
        Wukv_sb = big.tile([128, D], bf16, tag="Wuv")
        Wq_sb = big.tile([128, D // 128, D], bf16, tag="Wq")
        Wo_sb = big.tile([128, D // 128, D], bf16, tag="Wo")

        def dma_xT(n):
            nc.sync.dma_start(
                xT_sb[:, :, 512 * n:512 * (n + 1)],
                xT[:, 512 * n:512 * (n + 1)]
                .rearrange("(a p) s -> p a s", p=128))

        dma_xT(0)
        nc.sync.dma_start(Wdkv_sb[:],
                          Wdkv.rearrange("(a p) l -> p a l", p=128))
        dma_xT(1)
        nc.sync.dma_start(Wukv_sb[:], Wukv[:, D:2 * D])
        dma_xT(2)
        dma_xT(3)
        nc.sync.dma_start(Wq_sb[:, :, 0:128],
                          Wq[:, 0:128].rearrange("(a p) s -> p a s", p=128))
        WukT_sb = big.tile([128, D // 128, L], bf16, tag="WukT")
        nc.sync.dma_start(WukT_sb[:],
                          WukT.rearrange("(a p) l -> p a l", p=128))
        tw_sb = sm.tile([32, 128], bf16, tag="tw")
        nc.sync.dma_start(tw_sb[:], Twedge[:])
        i32_sb = sm.tile([32, 32], bf16, tag="i32")
        nc.sync.dma_start(i32_sb[:], I32[:])
        e2_sb = sm.tile([2, 128], f32r, tag="e2")
        nc.sync.dma_start(e2_sb[:], E2[:])
        ones_sb = sm.tile([1, 130], f32r, tag="ones")
        nc.sync.dma_start(ones_sb[:], Ones[:])
        bo_sb = sm.tile([1, D], f32r, tag="bo")
        nc.sync.dma_start(bo_sb[:], bo[:])
        nc.sync.dma_start(Wq_sb[:, :, 128:256],
                          Wq[:, 128:256].rearrange("(a p) s -> p a s", p=128))
        nc.sync.dma_start(Wq_sb[:, :, 256:D],
                          Wq[:, 256:D].rearrange("(a p) s -> p a s", p=128))
        nc.sync.dma_start(Wo_sb[:], Wo.rearrange("(a p) n -> p a n", p=128))

        latT_sb = big.tile([128, S], bf16, tag="latT")
        qT_sb = big.tile([128, H // 2, GQ], bf16, tag="qT")
        qL_sb = big.tile([128, H, GQ], bf16, tag="qL")
        va_sb = big.tile([128, NKT, H * AUG], bf16, tag="va")
        ctxT_sb = big.tile([128, H // 2, GQ], bf16, tag="ctxT")
        bob_sb = sm.tile([128, D], f32r, tag="bob")

        # queries = columns 0::4 of (host-rotated) xT
        xq = xT_sb.rearrange("p a (q four) -> p a four q", four=4)

        # ones column of v_aug (disjoint from the va copies; no dep)
        nc.any.memset(
            va_sb[:].rearrange("p u (h e) -> p u h e", e=AUG)[:, :, :, HD],
            1.0)

        # GPSIMD cannot access PSUM.  Prologue psum->sbuf copies alternate
        # DVE / ScalarE (exp hasn't started); in-attention copies go to DVE
        # so ScalarE is kept free for the exp stream.
        state = {"attn": False, "flip": 0}

        def copy(dst, src):
            if state["attn"]:
                nc.vector.tensor_copy(dst, src)
            elif state["flip"] % 2 == 0:
                nc.vector.tensor_copy(dst, src)
            else:
                nc.scalar.copy(dst, src)
            state["flip"] += 1

        # --------- background GEMM units (pout psums, [128,512]) -------
        def u_latT(n):
            def emit():
                ps = pout.tile([128, 512], f32, tag="p1", name=f"lat{n}")
                for k in range(D // 128):
                    nc.tensor.matmul(ps[:], Wdkv_sb[:, k, :],
                                     xT_sb[:, k, 512 * n:512 * (n + 1)],
                                     start=(k == 0), stop=(k == D // 128 - 1))
                copy(latT_sb[:, 512 * n:512 * (n + 1)], ps[:])
            return emit

        def u_qT(m):
            def emit():
                ps = pout.tile([128, 512], f32, tag="p1", name=f"q{m}")
                for k in range(D // 128):
                    nc.tensor.matmul(ps[:],
                                     Wq_sb[:, k, 128 * m:128 * (m + 1)],
                                     xq[:, k, 0, :],
                                     start=(k == 0), stop=(k == D // 128 - 1))
                copy(qT_sb[:, m, :], ps[:])
            return emit

        def u_qL(h):
            # project head h's queries into latent space: qL_h = Wuk_h^T q_h
            def emit():
                ps = pout.tile([128, 512], f32, tag="p1", name=f"ql{h}")
                p0 = 64 * (h % 2)
                nc.tensor.matmul(ps[:],
                                 WukT_sb[p0:p0 + 64, h // 2, :],
                                 qT_sb[p0:p0 + 64, h // 2, :],
                                 start=True, stop=True)
                copy(qL_sb[:, h, :], ps[:])
            return emit

        def u_va(u, half):
            def emit():
                ps = pout.tile([128, 512], f32, tag="p1",
                               name=f"v{u}_{half}")
                nc.tensor.matmul(
                    ps[:], latT_sb[:, 128 * u:128 * (u + 1)],
                    Wukv_sb[:, 512 * half:512 * (half + 1)],
                    start=True, stop=True)
                dst = va_sb[:, u, AUG * 8 * half:AUG * 8 * (half + 1)]
                copy(dst.rearrange("p (h e) -> p h e", e=AUG)[:, :, 0:HD],
                     ps[:].rearrange("p (h e) -> p h e", e=HD))
            return emit

        def u_bob():
            for hh in range(2):
                ps = pout.tile([128, 512], f32, tag="p1", name=f"bob{hh}")
                nc.tensor.matmul(ps[:], ones_sb[0:1, 0:128],
                                 bo_sb[0:1, 512 * hh:512 * (hh + 1)],
                                 start=True, stop=True)
                copy(bob_sb[:, 512 * hh:512 * (hh + 1)], ps[:])

        # ---------------- attention stream -----------------------------
        def emit_scores(hp, bi):
            bitems, fill = bins[bi]
            sps = psc.tile([128, 1024], f32, tag="sc", name=f"s{hp}_{bi}")
            for par in range(2):
                o0 = 512 * par
                h = 2 * hp + par
                for (u, cs, wedge), o in bitems:
                    sw = QT - cs
                    nc.tensor.matmul(
                        sps[:, o0 + o:o0 + o + sw],
                        latT_sb[:, KT * u:KT * (u + 1)],
                        qL_sb[:, h, cs:QT],
                        start=True, stop=not wedge)
                    if wedge:
                        wn = min(32, sw)
                        nc.tensor.matmul(sps[:, o0 + o:o0 + o + wn],
                                         tw_sb[:], i32_sb[:, 0:wn],
                                         start=False, stop=True)
            return sps

        def emit_exp(hp, bi, sps):
            bitems, fill = bins[bi]
            exps = sexp.tile([128, 1024], bf16, tag="exp",
                             name=f"e{hp}_{bi}")
            nc.scalar.activation(
                exps[:].rearrange("p (b c) -> p b c", b=2)[:, :, 0:fill],
                sps[:].rearrange("p (b c) -> p b c", b=2)[:, :, 0:fill],
                AF.Exp, scale=0.125)
            return exps

        cps_map = {}

        def emit_ctx(hp, bi, exps):
            bitems, fill = bins[bi]
            if hp not in cps_map:
                cps_map[hp] = ([pctx.tile([AUG, QT], f32, tag="ctx",
                                          name=f"c{hp}_{p}")
                                for p in range(2)], [0])
            cps, cnt = cps_map[hp]
            n_tot = len(items)
            for (u, cs, wedge), o in bitems:
                cnt[0] += 1
                for par in range(2):
                    h = hp * 2 + par
                    nc.tensor.matmul(
                        cps[par][:, cs:QT],
                        va_sb[:, u, AUG * h:AUG * (h + 1)],
                        exps[:, 512 * par + o:512 * par + o + (QT - cs)],
                        start=(cnt[0] == 1), stop=(cnt[0] == n_tot),
                        skip_group_check=True)
            if bi == NB - 1:
                _finish_a(hp, cps)
                del cps_map[hp]

        recs0_map = {}

        def _finish_a(hp, cps):
            # free cps fast: recip + parity copies; the reciprocal row is
            # DMA'd to partitions 0/1 now so the broadcast matmul
            # (_finish_b, emitted a few bins later) never stalls the PE.
            recs = precs.tile([65, 2 * QT], f32r, tag="recs",
                              name=f"recs{hp}")
            recs0 = precs.tile([2, QT], f32r, tag="recs0",
                               name=f"recs0_{hp}")
            for par in range(2):
                rc = slice(par * QT, (par + 1) * QT)
                with nc.allow_low_precision(
                        reason="f32r is a bit-identical f32 alias"):
                    nc.vector.reciprocal(recs[64:65, rc],
                                         cps[par][HD:HD + 1, :])
            nc.vector.tensor_copy(ctxT_sb[0:64, hp, :], cps[0][0:HD, :])
            st = sout.tile([64, GQ], bf16, tag="st")
            nc.vector.tensor_copy(st[:], cps[1][0:HD, :])
            nc.sync.dma_start(ctxT_sb[64:128, hp, :], st[:])
            nc.sync.dma_start(recs0[:], recs[64:65, :])
            recs0_map[hp] = recs0

        def _finish_b(hp):
            # rp[p, q] = recs0[p//64, q] via a rank-2 selector matmul; the
            # normalization reads the psum directly on DVE (no staging copy)
            recs0 = recs0_map.pop(hp)
            rp = pout.tile([128, GQ], f32, tag="p1", name=f"rp{hp}")
            nc.tensor.matmul(rp[:], e2_sb[:], recs0[:],
                             start=True, stop=True)
            nc.vector.tensor_tensor(ctxT_sb[:, hp, :], ctxT_sb[:, hp, :],
                                    rp[:], ALU.mult)

        # ------------- emission schedule -------------
        # Prologue uses the (still idle) 2-bank psc tiles so two 512-col
        # chunks share one psum tile and one copy: fewer, larger copies and
        # a 4-slot psum rotation instead of 2.
        def pro_latT(h):
            ps = psc.tile([128, 1024], f32, tag="sc", name=f"plat{h}")
            for nn in range(2):
                n = 2 * h + nn
                for k in range(D // 128):
                    nc.tensor.matmul(ps[:, 512 * nn:512 * (nn + 1)],
                                     Wdkv_sb[:, k, :],
                                     xT_sb[:, k, 512 * n:512 * (n + 1)],
                                     start=(k == 0), stop=(k == D // 128 - 1))
            copy(latT_sb[:, 1024 * h:1024 * (h + 1)], ps[:])

        def pro_qL01():
            ps = psc.tile([128, 1024], f32, tag="sc", name="pql")
            for h in range(2):
                p0 = 64 * h
                nc.tensor.matmul(ps[:, 512 * h:512 * (h + 1)],
                                 WukT_sb[p0:p0 + 64, 0, :],
                                 qT_sb[p0:p0 + 64, 0, :],
                                 start=True, stop=True)
            copy(qL_sb[:, 0:2, :],
                 ps[:].rearrange("p (h q) -> p h q", h=2))

        def pro_va(p):
            ps = psc.tile([128, 1024], f32, tag="sc", name=f"pv{p}")
            for uu in range(2):
                u = 2 * p + uu
                nc.tensor.matmul(
                    ps[:, 512 * uu:512 * (uu + 1)],
                    latT_sb[:, 128 * u:128 * (u + 1)],
                    Wukv_sb[:, 0:512],
                    start=True, stop=True)
            dst = va_sb[:, 2 * p:2 * p + 2, 0:AUG * 8]
            copy(dst.rearrange("p u (h e) -> p u h e", e=AUG)[:, :, :, 0:HD],
                 ps[:].rearrange("p (u h e) -> p u h e", u=2, e=HD))

        pro_latT(0)
        pro_latT(1)
        u_qT(0)()
        pro_qL01()
        for p in range(NKT // 2):
            pro_va(p)
        u_bob()
        state["attn"] = True

        # background units due during attention: (flat_bin_index, emit_fn)
        bg = []
        for j in range(1, 8):
            base = (j - 1) * NB
            bg.append((base + 2, u_qT(j)))
            bg.append((base + 5, u_qL(2 * j)))
            bg.append((base + 6, u_qL(2 * j + 1)))
        for k in range(NKT):
            bg.append((8 + 2 * k, u_va(k, 1)))
        for j in range(7):
            # normalization of head-pair j, a few bins into head-pair j+1
            bg.append((j * NB + NB + 4, lambda j=j: _finish_b(j)))
        bg.sort(key=lambda t: t[0])

        flat = [(hp, bi) for hp in range(H // 2) for bi in range(NB)]
        bgi = [0]
        pipe_sps = {}
        pipe_exps = {}

        def bg_drain(i):
            while bgi[0] < len(bg) and bg[bgi[0]][0] <= i:
                bg[bgi[0]][1]()
                bgi[0] += 1

        # per step i: scores(i+1) | exp(i) | bg GEMMs | ctx(i-2) — the bg
        # units sit between scores and ctx on the PE stream so the PE has
        # work while ScalarE finishes exp; ctx lags exp by two bins so the
        # PE never waits on a just-issued activation.
        pipe_sps[0] = emit_scores(*flat[0])
        for i in range(len(flat)):
            if i + 1 < len(flat):
                pipe_sps[i + 1] = emit_scores(*flat[i + 1])
            pipe_exps[i] = emit_exp(*flat[i], pipe_sps.pop(i))
            bg_drain(i)
            if i - 2 >= 0:
                emit_ctx(*flat[i - 2], pipe_exps.pop(i - 2))
        for i in (len(flat) - 2, len(flat) - 1):
            emit_ctx(*flat[i], pipe_exps.pop(i))
        _finish_b(7)

        # ---------------- output projection + bias ----------------
        for m in range(GQ // 128):
            for n in range(D // 512):
                ps = pout.tile([128, 512], f32, tag="p1", name=f"o{m}_{n}")
                for k in range(D // 128):
                    nc.tensor.matmul(
                        ps[:], ctxT_sb[:, k, 128 * m:128 * (m + 1)],
                        Wo_sb[:, k, 512 * n:512 * (n + 1)],
                        start=(k == 0), stop=(k == D // 128 - 1))
                ob = sout.tile([128, 512], f32, tag="ob")
                nc.vector.tensor_tensor(
                    ob[:], ps[:], bob_sb[:, 512 * n:512 * (n + 1)],
                    ALU.add)
                nc.sync.dma_start(
                    out[128 * m:128 * (m + 1), 512 * n:512 * (n + 1)], ob[:])


def _in_maps(x, offset, Wq, Wdkv, Wukv, Wo, bo):
    import ml_dtypes
    items = _worklist(offset)
    f32 = np.float32
    bff = ml_dtypes.bfloat16
    maps = []
    i32 = np.eye(32, dtype=bff)
    common = {
        "Wq": np.ascontiguousarray(Wq).astype(bff),
        "Wdkv": np.ascontiguousarray(Wdkv).astype(bff),
        "Wukv": np.ascontiguousarray(Wukv).astype(bff),
        "WukT": np.ascontiguousarray(np.asarray(Wukv)[:, :D].T).astype(bff),
        "Wo": np.ascontiguousarray(Wo).astype(bff),
        "bo": np.ascontiguousarray(bo, f32).reshape(1, D),
        "I32": i32,
        "E2": np.concatenate([
            np.concatenate([np.ones((1, 64)), np.zeros((1, 64))], axis=1),
            np.concatenate([np.zeros((1, 64)), np.ones((1, 64))], axis=1),
        ], dtype=f32),
        "Ones": np.ones((1, 130), f32),
    }
    for c in range(NCORES):
        b, g = c // 4, c % 4
        m = dict(common)
        # rotate x columns so this core's queries are columns 0::4:
        # core-local column 4*t+r holds global row 4*t + ((r+g) % 4).
        perm = (np.arange(S) // 4) * 4 + (np.arange(S) + g) % 4
        m["xT"] = np.ascontiguousarray(x[b][perm].T).astype(bff)
        m["Twedge"] = _wedge_matrix(g, offset, items).astype(bff)
        maps.append(m)
    return maps


def kernel(x, offset, Wq, Wdkv, Wukv, Wo, bo):
    from concourse.bass_utils import run_bass_kernel_spmd
    off = int(np.asarray(offset))
    if off not in _cache:
        _cache[off] = _build(off)
    nc = _cache[off]
    maps = _in_maps(np.asarray(x, np.float32), off, Wq, Wdkv, Wukv, Wo, bo)
    res = run_bass_kernel_spmd(nc, maps, list(range(NCORES)))
    outf = np.empty((B, S, D), np.float32)
    for c in range(NCORES):
        b, g = c // 4, c % 4
        outf[b, g::4, :] = res.results[c]["out"]
    return outf


# revision 50
# speedup vs baseline: 4.1682x; 1.0081x over previous
"""Multi-Head Latent Attention (naive MLA) on 8 Trainium2 NeuronCores.

Sharding: data-parallel over batch (2) x causal-balanced sequence-parallel
over queries (4-way interleave): core c handles batch b = c//4, query group
g = c%4 (global query rows g, g+4, g+8, ...).  Every core runs the identical
SPMD program; only the data differs.  To keep the program data-independent,
the host rotates each x-column group of 4 so the core's queries sit at
columns 0::4 (keys are therefore mildly permuted within each group of 4;
the wedge-mask matrix, also host data, accounts for that permutation).
No collectives: each core produces the full output rows for its queries.

All matmuls contract over the SBUF partition dim, so everything is kept
"transposed" (feature-major) end to end and no on-device transposes are
needed.  Scores never materialize K: queries are projected into latent
space instead (k_h.q_h = (Wuk_h latent).q_h = latent.(Wuk_h^T q_h)):
  latentT = Wdkv^T @ x^T                  [128, 2048]  bf16
  qT      = Wq^T @ xT[:, 0::4]            [1024, 512]  bf16
  qL_h    = Wuk_h^T @ qT_h                [128, 512] per head, bf16
  v_aug   = [latent @ Wuv | ones] per key tile          bf16
  scoresT: per head pair, one 2-bank psum [128 keys, 2x512]: parity p's
           scores (latT_u^T @ qL) in columns 512p..; causal wedge added by
           a rank-32 mask matmul; ONE fused exp per bin on ScalarE covers
           both parities (1/sqrt(hd) folded into the activation scale);
           softmax denom comes for free as row 64 of the ctx matmul (ones
           column of v_aug)
  ctxT_h  = v_aug^T @ expT                [65, 512] psum accum over key tiles
  out     = matmul(lhsT=ctxT tiles, rhs=Wo); bias added by the psum->sbuf
            copy (tensor_tensor with a broadcast bias tile)  -> [512, 1024]

The attention stream is software-pipelined (scores one bin ahead of exp,
exp two bins ahead of ctx) and the phase-1/2 GEMMs (qT / qL / v_aug
chunks) plus the deferred per-head normalizations are emitted BETWEEN
attention bins so the PE keeps running while ScalarE does exp; psum->sbuf
copies run on DVE (GPSIMD cannot touch PSUM; it does the SBUF-only
normalization multiplies instead).
PSUM: psc 2x[128,1024] for scores (4 banks), pctx 2x[65,512] ctx accum
(2 banks), pout 2x[128,512] for background GEMMs / bias / out-proj.
"""

import numpy as np

B, S, D, L, H = 2, 2048, 1024, 128, 16
HD = D // H        # 64
AUG = HD + 1       # 65 (v dims + ones column for softmax denominator)
NCORES = 8
GQ = S // 4        # 512 queries per core
QT = 512           # queries per tile (single tile)
KT = 128           # keys per key tile
NKT = S // KT      # 16
NEG = -640.0       # additive mask pre-exp-scale (x 1/8 -> -80)

_cache = {}


def _worklist(offset):
    """Strip list [(u, cs, wedge)]; identical across cores.

    Query column c = global row 4*c+g, position +offset.  cs (first
    computed column of the strip) uses the worst core (g=3) so strip
    shapes are core-independent; the wedge matrix (data) carries g.
    """
    items = []
    for u in range(NKT):
        lo = KT * u
        min_qpos = 0 + offset
        max_qpos = 4 * (QT - 1) + 3 + offset
        if lo + KT - 1 <= min_qpos:
            items.append((u, 0, False))      # fully allowed
        elif lo > max_qpos:
            continue                         # fully masked: skip
        else:
            cs = max(0, -((-(lo - 3 - offset)) // 4))
            assert 0 <= cs < QT
            items.append((u, cs, True))
    assert items and items[0][1] == 0, "first strip must cover col 0"
    return items


def _wedge_matrix(g, offset, items):
    """[32, 128] f32: T[m, j] = NEG where local key j is masked at strip
    col m.  Host rotates x columns so queries are 0::4; local key index j
    of a strip is global position lo + pi(j), pi(j) = 4*(j//4)+(j%4+g)%4.
    Masked iff lo+pi(j) > q_pos = 4*(cs+m)+g+offset, i.e. pi(j) > 4*m+r0,
    r0 = 4*cs+g+offset-lo (strip-independent; asserted).
    """
    r0s = set()
    for (u, cs, wedge) in items:
        if wedge:
            r0s.add(4 * cs + g + offset - KT * u)
    if not r0s:
        r0s = {g}
    assert len(r0s) == 1, f"non-uniform wedge r0 {r0s} (offset={offset})"
    r0 = r0s.pop()
    assert 0 <= r0 <= 127, r0
    j = np.arange(128)
    pi = 4 * (j // 4) + (j % 4 + g) % 4
    T = np.zeros((32, 128), np.float32)
    for m in range(32):
        T[m, :] = np.where(pi > 4 * m + r0, NEG, 0.0)
    return T


def _blocks_of(items):
    """Pack strips into per-parity psum bins of <=512 cols (one matmul's
    psum output can't cross a bank).  Returns [(list[(item, off)], fill)]."""
    bins = []
    cur, w = [], 0
    for it in items:
        sw = QT - it[1]
        if w + sw > 512:
            bins.append((cur, w))
            cur, w = [], 0
        cur.append((it, w))
        w += sw
    if cur:
        bins.append((cur, w))
    return bins


def _build(offset, reps=1):
    import concourse.bacc as bacc
    import concourse.tile as tile
    import concourse.mybir as mybir
    from contextlib import ExitStack

    f32r = mybir.dt.float32r
    bf16 = mybir.dt.bfloat16
    f32 = mybir.dt.float32

    nc = bacc.Bacc("TRN2", target_bir_lowering=False, debug=False,
                   num_devices=NCORES)
    xT = nc.dram_tensor("xT", [D, S], bf16, kind="ExternalInput").ap()
    Wq = nc.dram_tensor("Wq", [D, D], bf16, kind="ExternalInput").ap()
    Wdkv = nc.dram_tensor("Wdkv", [D, L], bf16, kind="ExternalInput").ap()
    Wukv = nc.dram_tensor("Wukv", [L, 2 * D], bf16, kind="ExternalInput").ap()
    WukT = nc.dram_tensor("WukT", [D, L], bf16, kind="ExternalInput").ap()
    Wo = nc.dram_tensor("Wo", [D, D], bf16, kind="ExternalInput").ap()
    bo = nc.dram_tensor("bo", [1, D], f32r, kind="ExternalInput").ap()
    Twedge = nc.dram_tensor("Twedge", [32, 128], bf16,
                            kind="ExternalInput").ap()
    I32 = nc.dram_tensor("I32", [32, 32], bf16, kind="ExternalInput").ap()
    E2 = nc.dram_tensor("E2", [2, 128], f32r, kind="ExternalInput").ap()
    Ones = nc.dram_tensor("Ones", [1, 130], f32r, kind="ExternalInput").ap()
    out = nc.dram_tensor("out", [GQ, D], f32, kind="ExternalOutput").ap()

    for _rep in range(reps):
        _emit_body(nc, tile, mybir, ExitStack, offset,
                   xT, Wq, Wdkv, Wukv, WukT, Wo, bo, Twedge, I32, E2, Ones,
                   out)

    nc.compile()
    return nc


def _emit_body(nc, tile, mybir, ExitStack, offset,
               xT, Wq, Wdkv, Wukv, WukT, Wo, bo, Twedge, I32, E2, Ones, out):
    f32r = mybir.dt.float32r
    bf16 = mybir.dt.bfloat16
    f32 = mybir.dt.float32
    AF = mybir.ActivationFunctionType
    ALU = mybir.AluOpType

    items = _worklist(offset)
    bins = _blocks_of(items)
    NB = len(bins)

    with tile.TileContext(nc) as tc, ExitStack() as ctx:
        big = ctx.enter_context(tc.tile_pool(name="big", bufs=1, side="left"))
        sm = ctx.enter_context(tc.tile_pool(name="sm", bufs=1, side="right"))
        sexp = ctx.enter_context(tc.tile_pool(name="sexp", bufs=4,
                                              side="right"))
        sout = ctx.enter_context(tc.tile_pool(name="sout", bufs=2,
                                              side="right"))
        precs = ctx.enter_context(tc.tile_pool(name="precs", bufs=2,
                                               side="right"))
        psc = ctx.enter_context(tc.tile_pool(name="psc", bufs=2,
                                             space="PSUM", side="left"))
        pctx = ctx.enter_context(tc.tile_pool(name="pctx", bufs=2,
                                              space="PSUM", side="right"))
        pout = ctx.enter_context(tc.tile_pool(name="pout", bufs=2,
                                              space="PSUM", side="right"))

        # ------------- input DMAs (dependency-priority order) -------------
        xT_sb = big.tile([128, D // 128, S], bf16, tag="xT")
        Wdkv_sb = big.tile([128, D // 128, L], bf16, tag="Wdkv")
        Wukv_sb = big.tile([128, D], bf16, tag="Wuv")
        Wq_sb = big.tile([128, D // 128, D], bf16, tag="Wq")
        Wo_sb = big.tile([128, D // 128, D], bf16, tag="Wo")

        def dma_xT(n):
            # split by k-half across two DGE queues: latT's first
            # accumulation chunks start sooner and queue setup overlaps
            for kk, eng in ((0, nc.sync), (1, nc.scalar)):
                eng.dma_start(
                    xT_sb[:, 4 * kk:4 * (kk + 1), 512 * n:512 * (n + 1)],
                    xT[512 * kk:512 * (kk + 1), 512 * n:512 * (n + 1)]
                    .rearrange("(a p) s -> p a s", p=128))

        dma_xT(0)
        nc.sync.dma_start(Wdkv_sb[:],
                          Wdkv.rearrange("(a p) l -> p a l", p=128))
        dma_xT(1)
        nc.sync.dma_start(Wukv_sb[:], Wukv[:, D:2 * D])
        dma_xT(2)
        dma_xT(3)
        nc.sync.dma_start(Wq_sb[:, :, 0:128],
                          Wq[:, 0:128].rearrange("(a p) s -> p a s", p=128))
        WukT_sb = big.tile([128, D // 128, L], bf16, tag="WukT")
        nc.sync.dma_start(WukT_sb[:],
                          WukT.rearrange("(a p) l -> p a l", p=128))
        tw_sb = sm.tile([32, 128], bf16, tag="tw")
        nc.sync.dma_start(tw_sb[:], Twedge[:])
        i32_sb = sm.tile([32, 32], bf16, tag="i32")
        nc.sync.dma_start(i32_sb[:], I32[:])
        e2_sb = sm.tile([2, 128], f32r, tag="e2")
        nc.sync.dma_start(e2_sb[:], E2[:])
        ones_sb = sm.tile([1, 130], f32r, tag="ones")
        nc.sync.dma_start(ones_sb[:], Ones[:])
        bo_sb = sm.tile([1, D], f32r, tag="bo")
        nc.sync.dma_start(bo_sb[:], bo[:])
        nc.sync.dma_start(Wq_sb[:, :, 128:256],
                          Wq[:, 128:256].rearrange("(a p) s -> p a s", p=128))
        nc.sync.dma_start(Wq_sb[:, :, 256:D],
                          Wq[:, 256:D].rearrange("(a p) s -> p a s", p=128))
        nc.sync.dma_start(Wo_sb[:], Wo.rearrange("(a p) n -> p a n", p=128))

        latT_sb = big.tile([128, S], bf16, tag="latT")
        qT_sb = big.tile([128, H // 2, GQ], bf16, tag="qT")
        qL_sb = big.tile([128, H, GQ], bf16, tag="qL")
        va_sb = big.tile([128, NKT, H * AUG], bf16, tag="va")
        ctxT_sb = big.tile([128, H // 2, GQ], bf16, tag="ctxT")
        bob_sb = sm.tile([128, D], f32r, tag="bob")

        # queries = columns 0::4 of (host-rotated) xT
        xq = xT_sb.rearrange("p a (q four) -> p a four q", four=4)

        # ones column of v_aug (disjoint from the va copies; no dep)
        nc.any.memset(
            va_sb[:].rearrange("p u (h e) -> p u h e", e=AUG)[:, :, :, HD],
            1.0)

        # GPSIMD cannot access PSUM.  Prologue psum->sbuf copies alternate
        # DVE / ScalarE (exp hasn't started); in-attention copies go to DVE
        # so ScalarE is kept free for the exp stream.
        state = {"attn": False, "flip": 0}

        def copy(dst, src):
            if state["attn"]:
                nc.vector.tensor_copy(dst, src)
            elif state["flip"] % 2 == 0:
                nc.vector.tensor_copy(dst, src)
            else:
                nc.scalar.copy(dst, src)
            state["flip"] += 1

        # --------- background GEMM units (pout psums, [128,512]) -------
        def u_latT(n):
            def emit():
                ps = pout.tile([128, 512], f32, tag="p1", name=f"lat{n}")
                for k in range(D // 128):
                    nc.tensor.matmul(ps[:], Wdkv_sb[:, k, :],
                                     xT_sb[:, k, 512 * n:512 * (n + 1)],
                                     start=(k == 0), stop=(k == D // 128 - 1))
                copy(latT_sb[:, 512 * n:512 * (n + 1)], ps[:])
            return emit

        def u_qT(m):
            def emit():
                ps = pout.tile([128, 512], f32, tag="p1", name=f"q{m}")
                for k in range(D // 128):
                    nc.tensor.matmul(ps[:],
                                     Wq_sb[:, k, 128 * m:128 * (m + 1)],
                                     xq[:, k, 0, :],
                                     start=(k == 0), stop=(k == D // 128 - 1))
                copy(qT_sb[:, m, :], ps[:])
            return emit

        def u_qL(h):
            # project head h's queries into latent space: qL_h = Wuk_h^T q_h
            def emit():
                ps = pout.tile([128, 512], f32, tag="p1", name=f"ql{h}")
                p0 = 64 * (h % 2)
                nc.tensor.matmul(ps[:],
                                 WukT_sb[p0:p0 + 64, h // 2, :],
                                 qT_sb[p0:p0 + 64, h // 2, :],
                                 start=True, stop=True)
                copy(qL_sb[:, h, :], ps[:])
            return emit

        def u_va(u, half):
            def emit():
                ps = pout.tile([128, 512], f32, tag="p1",
                               name=f"v{u}_{half}")
                nc.tensor.matmul(
                    ps[:], latT_sb[:, 128 * u:128 * (u + 1)],
                    Wukv_sb[:, 512 * half:512 * (half + 1)],
                    start=True, stop=True)
                dst = va_sb[:, u, AUG * 8 * half:AUG * 8 * (half + 1)]
                copy(dst.rearrange("p (h e) -> p h e", e=AUG)[:, :, 0:HD],
                     ps[:].rearrange("p (h e) -> p h e", e=HD))
            return emit

        def u_bob():
            for hh in range(2):
                ps = pout.tile([128, 512], f32, tag="p1", name=f"bob{hh}")
                nc.tensor.matmul(ps[:], ones_sb[0:1, 0:128],
                                 bo_sb[0:1, 512 * hh:512 * (hh + 1)],
                                 start=True, stop=True)
                copy(bob_sb[:, 512 * hh:512 * (hh + 1)], ps[:])

        # ---------------- attention stream -----------------------------
        def emit_scores(hp, bi):
            bitems, fill = bins[bi]
            sps = psc.tile([128, 1024], f32, tag="sc", name=f"s{hp}_{bi}")
            for par in range(2):
                o0 = 512 * par
                h = 2 * hp + par
                for (u, cs, wedge), o in bitems:
                    sw = QT - cs
                    nc.tensor.matmul(
                        sps[:, o0 + o:o0 + o + sw],
                        latT_sb[:, KT * u:KT * (u + 1)],
                        qL_sb[:, h, cs:QT],
                        start=True, stop=not wedge)
                    if wedge:
                        wn = min(32, sw)
                        nc.tensor.matmul(sps[:, o0 + o:o0 + o + wn],
                                         tw_sb[:], i32_sb[:, 0:wn],
                                         start=False, stop=True)
            return sps

        def emit_exp(hp, bi, sps):
            bitems, fill = bins[bi]
            exps = sexp.tile([128, 1024], bf16, tag="exp",
                             name=f"e{hp}_{bi}")
            nc.scalar.activation(
                exps[:].rearrange("p (b c) -> p b c", b=2)[:, :, 0:fill],
                sps[:].rearrange("p (b c) -> p b c", b=2)[:, :, 0:fill],
                AF.Exp, scale=0.125)
            return exps

        cps_map = {}

        def emit_ctx(hp, bi, exps):
            bitems, fill = bins[bi]
            if hp not in cps_map:
                cps_map[hp] = ([pctx.tile([AUG, QT], f32, tag="ctx",
                                          name=f"c{hp}_{p}")
                                for p in range(2)], [0])
            cps, cnt = cps_map[hp]
            n_tot = len(items)
            for (u, cs, wedge), o in bitems:
                cnt[0] += 1
                for par in range(2):
                    h = hp * 2 + par
                    nc.tensor.matmul(
                        cps[par][:, cs:QT],
                        va_sb[:, u, AUG * h:AUG * (h + 1)],
                        exps[:, 512 * par + o:512 * par + o + (QT - cs)],
                        start=(cnt[0] == 1), stop=(cnt[0] == n_tot),
                        skip_group_check=True)
            if bi == NB - 1:
                _finish_a(hp, cps)
                del cps_map[hp]

        recs0_map = {}

        def _finish_a(hp, cps):
            # free cps fast: recip + parity copies; the reciprocal row is
            # DMA'd to partitions 0/1 now so the broadcast matmul
            # (_finish_b, emitted a few bins later) never stalls the PE.
            recs = precs.tile([65, 2 * QT], f32r, tag="recs",
                              name=f"recs{hp}")
            recs0 = precs.tile([2, QT], f32r, tag="recs0",
                               name=f"recs0_{hp}")
            for par in range(2):
                rc = slice(par * QT, (par + 1) * QT)
                with nc.allow_low_precision(
                        reason="f32r is a bit-identical f32 alias"):
                    nc.vector.reciprocal(recs[64:65, rc],
                                         cps[par][HD:HD + 1, :])
            nc.vector.tensor_copy(ctxT_sb[0:64, hp, :], cps[0][0:HD, :])
            st = sout.tile([64, GQ], bf16, tag="st")
            nc.vector.tensor_copy(st[:], cps[1][0:HD, :])
            nc.sync.dma_start(ctxT_sb[64:128, hp, :], st[:])
            nc.sync.dma_start(recs0[:], recs[64:65, :])
            recs0_map[hp] = recs0

        def _finish_b(hp):
            # rp[p, q] = recs0[p//64, q] via a rank-2 selector matmul; the
            # normalization reads the psum directly on DVE (no staging copy)
            recs0 = recs0_map.pop(hp)
            rp = pout.tile([128, GQ], f32, tag="p1", name=f"rp{hp}")
            nc.tensor.matmul(rp[:], e2_sb[:], recs0[:],
                             start=True, stop=True)
            nc.vector.tensor_tensor(ctxT_sb[:, hp, :], ctxT_sb[:, hp, :],
                                    rp[:], ALU.mult)

        # ------------- emission schedule -------------
        # Prologue uses the (still idle) 2-bank psc tiles so two 512-col
        # chunks share one psum tile and one copy: fewer, larger copies and
        # a 4-slot psum rotation instead of 2.
        def pro_latT(h):
            ps = psc.tile([128, 1024], f32, tag="sc", name=f"plat{h}")
            for nn in range(2):
                n = 2 * h + nn
                for k in range(D // 128):
                    nc.tensor.matmul(ps[:, 512 * nn:512 * (nn + 1)],
                                     Wdkv_sb[:, k, :],
                                     xT_sb[:, k, 512 * n:512 * (n + 1)],
                                     start=(k == 0), stop=(k == D // 128 - 1))
            copy(latT_sb[:, 1024 * h:1024 * (h + 1)], ps[:])

        def pro_qL01():
            ps = psc.tile([128, 1024], f32, tag="sc", name="pql")
            for h in range(2):
                p0 = 64 * h
                nc.tensor.matmul(ps[:, 512 * h:512 * (h + 1)],
                                 WukT_sb[p0:p0 + 64, 0, :],
                                 qT_sb[p0:p0 + 64, 0, :],
                                 start=True, stop=True)
            copy(qL_sb[:, 0:2, :],
                 ps[:].rearrange("p (h q) -> p h q", h=2))

        def pro_va(p):
            ps = psc.tile([128, 1024], f32, tag="sc", name=f"pv{p}")
            for uu in range(2):
                u = 2 * p + uu
                nc.tensor.matmul(
                    ps[:, 512 * uu:512 * (uu + 1)],
                    latT_sb[:, 128 * u:128 * (u + 1)],
                    Wukv_sb[:, 0:512],
                    start=True, stop=True)
            dst = va_sb[:, 2 * p:2 * p + 2, 0:AUG * 8]
            copy(dst.rearrange("p u (h e) -> p u h e", e=AUG)[:, :, :, 0:HD],
                 ps[:].rearrange("p (u h e) -> p u h e", u=2, e=HD))

        pro_latT(0)
        pro_latT(1)
        u_qT(0)()
        pro_qL01()
        for p in range(NKT // 2):
            pro_va(p)
        state["attn"] = True

        # background units due during attention: (flat_bin_index, emit_fn)
        bg = []
        for j in range(1, 8):
            base = (j - 1) * NB
            bg.append((base + 2, u_qT(j)))
            bg.append((base + 5, u_qL(2 * j)))
            bg.append((base + 6, u_qL(2 * j + 1)))
        for k in range(NKT):
            bg.append((8 + 2 * k, u_va(k, 1)))
        for j in range(7):
            # normalization of head-pair j, a few bins into head-pair j+1
            bg.append((j * NB + NB + 4, lambda j=j: _finish_b(j)))
        # bias broadcast is only needed by the out-proj tail: emit it in
        # the last head-pair's (otherwise draining) bins
        bg.append((7 * NB + 6, u_bob))
        bg.sort(key=lambda t: t[0])

        flat = [(hp, bi) for hp in range(H // 2) for bi in range(NB)]
        bgi = [0]
        pipe_sps = {}
        pipe_exps = {}

        def bg_drain(i):
            while bgi[0] < len(bg) and bg[bgi[0]][0] <= i:
                bg[bgi[0]][1]()
                bgi[0] += 1

        # per step i: scores(i+1) | exp(i) | bg GEMMs | ctx(i-2) — the bg
        # units sit between scores and ctx on the PE stream so the PE has
        # work while ScalarE finishes exp; ctx lags exp by two bins so the
        # PE never waits on a just-issued activation.
        pipe_sps[0] = emit_scores(*flat[0])
        for i in range(len(flat)):
            if i + 1 < len(flat):
                pipe_sps[i + 1] = emit_scores(*flat[i + 1])
            pipe_exps[i] = emit_exp(*flat[i], pipe_sps.pop(i))
            bg_drain(i)
            if i - 2 >= 0:
                emit_ctx(*flat[i - 2], pipe_exps.pop(i - 2))
        for i in (len(flat) - 2, len(flat) - 1):
            emit_ctx(*flat[i], pipe_exps.pop(i))
        _finish_b(7)

        # ---------------- output projection + bias ----------------
        for m in range(GQ // 128):
            for n in range(D // 512):
                ps = pout.tile([128, 512], f32, tag="p1", name=f"o{m}_{n}")
                for k in range(D // 128):
                    nc.tensor.matmul(
                        ps[:], ctxT_sb[:, k, 128 * m:128 * (m + 1)],
                        Wo_sb[:, k, 512 * n:512 * (n + 1)],
                        start=(k == 0), stop=(k == D // 128 - 1))
                ob = sout.tile([128, 512], f32, tag="ob")
                nc.vector.tensor_tensor(
                    ob[:], ps[:], bob_sb[:, 512 * n:512 * (n + 1)],
                    ALU.add)
                nc.sync.dma_start(
                    out[128 * m:128 * (m + 1), 512 * n:512 * (n + 1)], ob[:])


def _in_maps(x, offset, Wq, Wdkv, Wukv, Wo, bo):
    import ml_dtypes
    items = _worklist(offset)
    f32 = np.float32
    bff = ml_dtypes.bfloat16
    maps = []
    i32 = np.eye(32, dtype=bff)
    common = {
        "Wq": np.ascontiguousarray(Wq).astype(bff),
        "Wdkv": np.ascontiguousarray(Wdkv).astype(bff),
        "Wukv": np.ascontiguousarray(Wukv).astype(bff),
        "WukT": np.ascontiguousarray(np.asarray(Wukv)[:, :D].T).astype(bff),
        "Wo": np.ascontiguousarray(Wo).astype(bff),
        "bo": np.ascontiguousarray(bo, f32).reshape(1, D),
        "I32": i32,
        "E2": np.concatenate([
            np.concatenate([np.ones((1, 64)), np.zeros((1, 64))], axis=1),
            np.concatenate([np.zeros((1, 64)), np.ones((1, 64))], axis=1),
        ], dtype=f32),
        "Ones": np.ones((1, 130), f32),
    }
    for c in range(NCORES):
        b, g = c // 4, c % 4
        m = dict(common)
        # rotate x columns so this core's queries are columns 0::4:
        # core-local column 4*t+r holds global row 4*t + ((r+g) % 4).
        perm = (np.arange(S) // 4) * 4 + (np.arange(S) + g) % 4
        m["xT"] = np.ascontiguousarray(x[b][perm].T).astype(bff)
        m["Twedge"] = _wedge_matrix(g, offset, items).astype(bff)
        maps.append(m)
    return maps


def kernel(x, offset, Wq, Wdkv, Wukv, Wo, bo):
    from concourse.bass_utils import run_bass_kernel_spmd
    off = int(np.asarray(offset))
    if off not in _cache:
        _cache[off] = _build(off)
    nc = _cache[off]
    maps = _in_maps(np.asarray(x, np.float32), off, Wq, Wdkv, Wukv, Wo, bo)
    res = run_bass_kernel_spmd(nc, maps, list(range(NCORES)))
    outf = np.empty((B, S, D), np.float32)
    for c in range(NCORES):
        b, g = c // 4, c % 4
        outf[b, g::4, :] = res.results[c]["out"]
    return outf
